# revision 2
# baseline (speedup 1.0000x reference)
"""DNDT forward kernel v2 for Trainium2 (8 NeuronCores, data-parallel).

Math (matches the reference):
    w = [1,2,3,4];  b = [0, cumsum(-sort(beta))]
    sigma[i,f,k] = sigmoid((x[i,f]*w[k] + b[k]) / T)            [B, 6, 4]
    leaves[i]    = kron(sigma[i,0], ..., sigma[i,5])            [B, 4096]
    out          = leaves @ L                                   [B, 10]

Restructured:
    A[i,a]  = kron(s0, s1)          a = k0*4+k1      in [0,16)
    Bm[i,b] = kron(s2, s3, s4, s5)  b-col order (vv, uu)
    M[i,(a,c)] = sum_b Bm[i,b] * L3[b, (a,c)]   (PE matmul, K=256)
    out[i,c] = sum_a A[i,a] * M[i,(a,c)]        (DVE prod + tree)

Host sends x4w[i,(f,k)] = x[i,f]*w_k/T (fp16) so the device lead-in is
one add + sigmoid.  Per-core layout: 8192 rows = 4 supertiles of
[128 partitions x G=16 rows]; PE/evac/combine pipelined per half-supertile.
"""

import os
import numpy as np

import concourse.bacc as bacc
import concourse.mybir as mybir
import concourse.tile as tile
from concourse.bass_utils import run_bass_kernel_spmd

F32 = mybir.dt.float32
F16 = mybir.dt.float16
U32 = mybir.dt.uint32

B, F, NB, NCLS = 65536, 6, 4, 10
CORES = 8
ROWS = B // CORES          # 8192
G = 16                     # rows per partition per supertile
ST_ROWS = 128 * G          # 2048
N_ST = ROWS // ST_ROWS     # 4
HQ = G // 2                # 8 q per half-supertile
TEMP = 0.1

_NC_CACHE = {}

COMBINE = os.environ.get("K_COMBINE", "A")   # A: scalar-evac + prod2x | B: prod from PSUM 1x


def _build_nc():
    nc = bacc.Bacc("TRN2", target_bir_lowering=False, debug=False)

    xw = nc.dram_tensor("xw", [ROWS, F * NB], F16, kind="ExternalInput")
    bt = nc.dram_tensor("bt", [128, F * NB], F16, kind="ExternalInput")
    ident = nc.dram_tensor("ident", [128, 128], F16, kind="ExternalInput")
    l3p = nc.dram_tensor("l3p", [128, 2, 160], F16, kind="ExternalInput")
    outc = nc.dram_tensor("outc", [ROWS, NCLS], F32, kind="ExternalOutput")

    with tile.TileContext(nc) as tc:
        with (
            tc.tile_pool(name="consts", bufs=1) as consts,
            tc.tile_pool(name="io", bufs=3) as io,
            tc.tile_pool(name="work", bufs=2) as work,
            tc.tile_pool(name="wts", bufs=3) as wts,
            tc.tile_pool(name="ps_t", bufs=2, space="PSUM") as ps_t,
            tc.tile_pool(name="ps_m", bufs=1, space="PSUM") as ps_m,
        ):
            bt_sb = consts.tile([128, 24], F16)
            nc.sync.dma_start(bt_sb[:, :], bt[:, :])
            id_sb = consts.tile([128, 128], F16)
            nc.sync.dma_start(id_sb[:, :], ident[:, :])
            l3_sb = consts.tile([128, 2, 160], F16)
            nc.sync.dma_start(l3_sb[:, :, :], l3p[:, :, :])

            bm_tiles = {}
            ap2_tiles = {}

            def front(st):
                base = st * ST_ROWS
                xs = xw[base:base + ST_ROWS, :].rearrange("(p g) fk -> p g fk", g=G)
                x_sb = io.tile([128, G, 24], F16, tag="x")
                nc.sync.dma_start(x_sb[:, :, :], xs)

                # z = x*w/T + b/T   [128, G, 24]  (pool)
                z = work.tile([128, G, 24], F16, tag="z")
                nc.gpsimd.tensor_add(
                    z[:, :, :], x_sb[:, :, :],
                    bt_sb[:, :].unsqueeze(1).broadcast_to((128, G, 24)))

                # sigma = sigmoid(z)  [128, G, 24]  (scalar)
                sig = work.tile([128, G, 24], F16, tag="sig")
                nc.scalar.activation(
                    sig[:, :, :], z[:, :, :],
                    mybir.ActivationFunctionType.Sigmoid)

                # auv[p, g, w, i, j] = sig[g, 2w, i] * sig[g, 2w+1, j]
                #   w=0: A (s0 x s1), w=1: u (s2 x s3), w=2: v (s4 x s5)  (pool)
                # Flat-base views so (g,w) merges into one AP dim (pool ops
                # support at most 3 free dims).
                auv = work.tile([128, G, 3, 16], F16, tag="auv")
                sgp = sig[:, :, :].rearrange("p g fk -> p (g fk)") \
                                  .rearrange("p (gw hk) -> p gw hk", hk=8)
                nc.gpsimd.tensor_mul(
                    auv[:, :, :, :].rearrange("p g w i -> p (g w i)")
                       .rearrange("p (gw i j) -> p gw i j", i=NB, j=NB),
                    sgp[:, :, 0:4].unsqueeze(3).broadcast_to((128, G * 3, 4, 4)),
                    sgp[:, :, 4:8].unsqueeze(2).broadcast_to((128, G * 3, 4, 4)),
                )

                # pair-dup A and v (own contiguous tiles so (g,i) AP dims merge:
                # walrus caps compute-op APs at 3 free dims)  (DVE 4x copies)
                ap2 = work.tile([128, G, 16, 2], F16, tag="ap2")
                nc.vector.tensor_copy(
                    ap2[:, :, :, :],
                    auv[:, :, 0, :].unsqueeze(3).broadcast_to((128, G, 16, 2)),
                )
                vp2 = work.tile([128, G, 16, 2], F16, tag="vp2")
                nc.vector.tensor_copy(
                    vp2[:, :, :, :],
                    auv[:, :, 2, :].unsqueeze(3).broadcast_to((128, G, 16, 2)),
                )

                # bm[p, g, vv, jp, t] = u[jp*2+t] * v[vv]   (DVE one-op 2x, 4-dim APs)
                bm = work.tile([128, G, 256], F16, tag="bm")
                nc.vector.tensor_mul(
                    bm[:, :, :].rearrange("p g (i j t) -> p g i j t", j=8, t=2),
                    auv[:, :, 1, :].rearrange("p g (j t) -> p g j t", t=2)
                        .unsqueeze(2).broadcast_to((128, G, 16, 8, 2)),
                    vp2[:, :, :, :].unsqueeze(3).broadcast_to((128, G, 16, 8, 2)),
                )

                bm_tiles[st] = bm
                ap2_tiles[st] = ap2

            def back(st):
                base = st * ST_ROWS
                bm = bm_tiles.pop(st)
                ap2 = ap2_tiles.pop(st)
                oq = io.tile([128, G, NCLS], F32, tag="oq")

                # both transpose groups first (ps_t holds 2), then both evacs
                # back-to-back on scalar, so neither engine head-of-line blocks
                tps, bmts = [], []
                for h in range(2):
                    q0 = h * HQ
                    tp = ps_t.tile([128, HQ, 256], F16, tag="tp")
                    for qq in range(HQ):
                        q = q0 + qq
                        nc.tensor.transpose(
                            tp[:, qq, 0:128], bm[:, q, 0:128], id_sb[:, :])
                        nc.tensor.transpose(
                            tp[:, qq, 128:256], bm[:, q, 128:256], id_sb[:, :])
                    tps.append(tp)
                for h in range(2):
                    bmt = wts.tile([128, HQ, 256], F16, tag="bmt")
                    nc.scalar.copy(
                        bmt[:, :, :].bitcast(U32), tps[h][:, :, :].bitcast(U32))
                    bmts.append(bmt)

                for h in range(2):
                    q0 = h * HQ
                    bmt = bmts[h]
                    mps = ps_m.tile([128, HQ, 160], F32, tag="m")
                    for qq in range(HQ):
                        nc.tensor.matmul(
                            mps[:, qq, :], bmt[:, qq, 0:128], l3_sb[:, 0, :],
                            start=True, stop=False)
                        nc.tensor.matmul(
                            mps[:, qq, :], bmt[:, qq, 128:256], l3_sb[:, 1, :],
                            start=False, stop=True)

                    gs = slice(q0, q0 + HQ)
                    if COMBINE == "A":
                        # evac M to fp16 (scalar/pool alternating), prod @2x (DVE)
                        msb = work.tile([128, HQ, 160], F16, tag="msb")
                        nc.scalar.copy(msb[:, :, :], mps[:, :, :])
                        prod = work.tile([128, HQ, 160], F16, tag="prod")
                        nc.vector.tensor_mul(
                            prod[:, :, :].rearrange("p g (a cp t) -> p g a cp t", cp=5, t=2),
                            ap2[:, gs, :, :].unsqueeze(3)
                                .broadcast_to((128, HQ, 16, 5, 2)),
                            msb[:, :, :].rearrange("p g (a cp t) -> p g a cp t", cp=5, t=2),
                        )
                    else:
                        # prod directly from PSUM (DVE 1x), paired-A APs, fp16 out
                        prod = work.tile([128, HQ, 160], F16, tag="prod")
                        nc.vector.tensor_mul(
                            prod[:, :, :].rearrange("p g (a cp t) -> p g a cp t", cp=5, t=2),
                            ap2[:, gs, :, :].unsqueeze(3)
                                .broadcast_to((128, HQ, 16, 5, 2)),
                            mps[:, :, :].rearrange("p g (a cp t) -> p g a cp t", cp=5, t=2),
                        )

                    # tree reduce over a: 160 -> 80 -> 40 -> 20 -> 10
                    f1 = work.tile([128, HQ, 80], F16, tag="f1")
                    nc.vector.tensor_add(f1[:, :, :], prod[:, :, 0:80], prod[:, :, 80:160])
                    f2 = work.tile([128, HQ, 40], F16, tag="f2")
                    nc.vector.tensor_add(f2[:, :, :], f1[:, :, 0:40], f1[:, :, 40:80])
                    f3 = work.tile([128, HQ, 20], F16, tag="f3")
                    nc.gpsimd.tensor_add(f3[:, :, :], f2[:, :, 0:20], f2[:, :, 20:40])
                    nc.gpsimd.tensor_add(
                        oq[:, gs, :], f3[:, :, 0:10], f3[:, :, 10:20])

                od = outc[base:base + ST_ROWS, :].rearrange("(p g) c -> p g c", g=G)
                nc.sync.dma_start(od, oq[:, :, :])

            for st in range(N_ST + 1):
                if st < N_ST:
                    front(st)
                if st >= 1:
                    back(st - 1)

    nc.compile()
    return nc


def _host_prep(x, beta, leaves2classes):
    x = np.asarray(x, dtype=np.float32)
    beta = np.asarray(beta, dtype=np.float32)
    L = np.asarray(leaves2classes, dtype=np.float32)

    w = np.linspace(1.0, float(NB), NB, dtype=np.float32)
    bs = np.sort(beta)
    b = np.concatenate([np.zeros(1, np.float32), np.cumsum(-bs, dtype=np.float32)])

    # x4w[i, (f,k)] = x[i,f] * w_k / T
    x4w = (x[:, :, None] * (w / TEMP)[None, None, :]).reshape(B, F * NB).astype(np.float16)
    bt24 = np.tile(b / np.float32(TEMP), F).astype(np.float16)
    BT = np.ascontiguousarray(np.broadcast_to(bt24, (128, F * NB)))

    # bm col order: j = vv*16 + jp*2 + t,  uu = jp*2+t, b_leaf = uu*16+vv
    # L3[b_col j, (a,c)] = L[a*256 + uu*16 + vv, c]; chunks j = chunk*128 + p
    j = np.arange(256)
    vv = j // 16
    uu = j % 16
    lrow = uu * 16 + vv                       # leaf sub-index (k2..k5)
    L3 = L.reshape(16, 256, NCLS)             # [a, (uu*16+vv), c]
    L3 = L3[:, lrow, :]                       # [a, j, c]
    L3 = np.transpose(L3, (1, 0, 2)).reshape(256, 16 * NCLS)
    L3P = np.ascontiguousarray(
        L3.reshape(2, 128, 16 * NCLS).transpose(1, 0, 2)).astype(np.float16)

    ident = np.eye(128, dtype=np.float16)
    return x4w, BT, ident, L3P


def kernel(x, beta, leaves2classes):
    x4w, BT, ident, L3P = _host_prep(x, beta, leaves2classes)

    if "nc" not in _NC_CACHE:
        _NC_CACHE["nc"] = _build_nc()
    nc = _NC_CACHE["nc"]

    in_maps = []
    for c in range(CORES):
        in_maps.append({
            "xw": np.ascontiguousarray(x4w[c * ROWS:(c + 1) * ROWS]),
            "bt": BT,
            "ident": ident,
            "l3p": L3P,
        })
    res = run_bass_kernel_spmd(nc, in_maps, core_ids=list(range(CORES)))
    out = np.concatenate([r["outc"] for r in res.results], axis=0)
    return out.astype(np.float32)


def make_in_maps(inputs):
    x4w, BT, ident, L3P = _host_prep(**inputs)
    return [{
        "xw": np.ascontiguousarray(x4w[c * ROWS:(c + 1) * ROWS]),
        "bt": BT,
        "ident": ident,
        "l3p": L3P,
    } for c in range(CORES)]
